# revision 1
# baseline (speedup 1.0000x reference)
"""MoE (DbrxExperts) expert-parallel Trainium2 kernel.

Strategy:
  - Host: compute per-(expert,token) combine weights cw from top_experts /
    top_weights, gather each expert's routed tokens, pad to a common
    capacity C, and pre-transpose operands so the device kernel needs no
    on-chip transposes.
  - Device (8 cores, SPMD, 2 experts/core): per expert
        gate_T = W1T_blocks^T @ XT     [F, C]   (contract H)
        up_T   = V1T_blocks^T @ XT     [F, C]
        hact_T = silu(gate_T) * up_T   [F, C]   (ACT + DVE)
        down   = hact_T_blocks^T @ W2  [C, H]   (contract F)
    All fp32. Output in natural [C, H] layout.
  - Host: out[tokens_e] += down_e * cw_e  (scaling folded into combine).
"""

import numpy as np
from contextlib import ExitStack

N_CORES = 8
B, S, H = 4, 2048, 1024
F, E = 2048, 16
T = B * S
E_LOC = E // N_CORES  # experts per core

P = 128
HT = H // P   # 8  h-tiles
FT = F // P   # 16 f-tiles

TRACE = False          # test.py sets this for profiled runs
TRACE_CORES = [7]      # core-0 NTFF capture crashes fast kernels here
MM_DTYPE = "fp32r"     # "fp32" | "fp32r" (tf32-rate, 4x PE) | "bf16"
LAST_RESULT = None     # BassKernelResults of last run (for test.py)

_nc_cache = {}


def _chunks(C):
    """Balanced c-chunks of <=1024 (each a multiple of 128), split into
    two matmul parts each <=512 and >=256 where possible (fp32r needs
    N>=256 for full rate)."""
    n = -(-C // 1024)
    base = (C // n) // P * P
    sizes = [base] * n
    rem = C - base * n
    i = 0
    while rem > 0:
        sizes[i] += P
        rem -= P
        i = (i + 1) % n
    out = []
    c0 = 0
    for s in sizes:
        if s > 512:
            p0 = (s // 2 + 63) // 64 * 64
            parts = [p0, s - p0]
        else:
            parts = [s]
        out.append((c0, s, parts))
        c0 += s
    return out


def _build_nc(C):
    # NOTE: reads module-global MM_DTYPE
    import concourse.tile as tile
    from concourse import bacc, mybir

    nc = bacc.Bacc("TRN2", target_bir_lowering=False, debug=False,
                   enable_asserts=False, num_devices=N_CORES)
    dt = mybir.dt.float32
    mdt = {"fp32": mybir.dt.float32, "fp32r": mybir.dt.float32r,
           "bf16": mybir.dt.bfloat16}[MM_DTYPE]
    mm = lambda ap: ap
    SILU = mybir.ActivationFunctionType.Silu

    xt = nc.dram_tensor("xt", [E_LOC, H, C], mdt, kind="ExternalInput").ap()
    # w1t/v1t arrive pre-blocked: [e, ft, p(h%128), o(h//128), f] so each
    # (e, ft) slice is contiguous and DMAs as 128 x 4KB descriptors
    w1t = nc.dram_tensor("w1t", [E_LOC, FT, P, HT, P], mdt,
                         kind="ExternalInput").ap()
    v1t = nc.dram_tensor("v1t", [E_LOC, FT, P, HT, P], mdt,
                         kind="ExternalInput").ap()
    w2 = nc.dram_tensor("w2", [E_LOC, F, H], mdt, kind="ExternalInput").ap()
    y = nc.dram_tensor("y", [E_LOC, C, H], dt, kind="ExternalOutput").ap()

    with tile.TileContext(nc) as tc:
        with ExitStack() as ctx:
            xt_pool = ctx.enter_context(tc.tile_pool(name="xt", bufs=HT))
            wst_pool = ctx.enter_context(tc.tile_pool(name="wst", bufs=4))
            w2_pool = ctx.enter_context(tc.tile_pool(name="w2sb", bufs=FT))
            hact_pool = ctx.enter_context(tc.tile_pool(name="hact", bufs=FT))
            silu_pool = ctx.enter_context(tc.tile_pool(name="silu", bufs=2))
            out_pool = ctx.enter_context(tc.tile_pool(name="out", bufs=2))
            ps_pool = ctx.enter_context(tc.tile_pool(name="ps", bufs=8, space="PSUM"))

            for e in range(E_LOC):
                # W2 tiles for this expert are emitted after the first
                # chunk's GEMM1/2 so their DMAs don't delay the XT/W1T
                # loads the first matmuls depend on
                w2_sb = []

                for ci, (c0, S_, parts) in enumerate(_chunks(C)):
                    # XT chunk: 8 tiles [128, S_], partition = h within tile
                    xt_sb = []
                    for ht in range(HT):
                        t = xt_pool.tile([P, S_], mdt, tag="xt")
                        nc.sync.dma_start(
                            t[:], xt[e, ht * P:(ht + 1) * P, c0:c0 + S_])
                        xt_sb.append(t)

                    # GEMM1/2 + GLU -> hact_T tiles [128, S_] per f-tile
                    hact_sb = []
                    for ft in range(FT):
                        h_t = hact_pool.tile([P, S_], mdt, tag="hact")
                        w1s = wst_pool.tile([P, HT, P], mdt, tag="wst")
                        v1s = wst_pool.tile([P, HT, P], mdt, tag="wst")
                        nc.sync.dma_start(w1s[:], w1t[e, ft])
                        nc.sync.dma_start(v1s[:], v1t[e, ft])
                        # all parts of this f-tile live at once so each
                        # LDWEIGHTS serves len(parts)*1 matmuls per matrix
                        offs = []
                        o = 0
                        for p_ in parts:
                            offs.append((o, p_))
                            o += p_
                        g_tiles = [ps_pool.tile([P, p_], dt, tag="ps",
                                                name=f"g{i_}")
                                   for i_, (_, p_) in enumerate(offs)]
                        u_tiles = [ps_pool.tile([P, p_], dt, tag="ps",
                                               name=f"u{i_}")
                                   for i_, (_, p_) in enumerate(offs)]
                        for ht in range(HT):
                            for i_, (o_, p_) in enumerate(offs):
                                nc.tensor.matmul(
                                    g_tiles[i_][:], mm(w1s[:, ht, :]),
                                    mm(xt_sb[ht][:, o_:o_ + p_]),
                                    start=(ht == 0), stop=(ht == HT - 1))
                            for i_, (o_, p_) in enumerate(offs):
                                nc.tensor.matmul(
                                    u_tiles[i_][:], mm(v1s[:, ht, :]),
                                    mm(xt_sb[ht][:, o_:o_ + p_]),
                                    start=(ht == 0), stop=(ht == HT - 1))
                        for i_, (o_, p_) in enumerate(offs):
                            sl = silu_pool.tile([P, p_], dt, tag="sl")
                            nc.scalar.activation(sl[:], g_tiles[i_][:], SILU)
                            nc.vector.tensor_mul(
                                h_t[:, o_:o_ + p_], sl[:], u_tiles[i_][:])
                        hact_sb.append(h_t)

                    if ci == 0:
                        for ft in range(FT):
                            t = w2_pool.tile([P, H], mdt, tag="w2",
                                             name=f"w2_{ft}")
                            nc.sync.dma_start(
                                t[:], w2[e, ft * P:(ft + 1) * P, :])
                            w2_sb.append(t)

                    # GEMM3: down[c, h] accumulated over f-tiles; H split
                    # into two 512 halves so psum slots stay one bank each
                    for ct in range(S_ // P):
                        o_t = out_pool.tile([P, H], dt, tag="o")
                        for hi, hh in enumerate(range(0, H, 512)):
                            d_ps = ps_pool.tile([P, 512], dt, tag="ps",
                                                name=f"d{hi}")
                            for ft in range(FT):
                                nc.tensor.matmul(
                                    d_ps[:],
                                    mm(hact_sb[ft][:, ct * P:(ct + 1) * P]),
                                    mm(w2_sb[ft][:, hh:hh + 512]),
                                    start=(ft == 0), stop=(ft == FT - 1))
                            nc.any.tensor_copy(o_t[:, hh:hh + 512], d_ps[:])
                            nc.sync.dma_start(
                                y[e, c0 + ct * P:c0 + (ct + 1) * P,
                                  hh:hh + 512],
                                o_t[:, hh:hh + 512])
    nc.compile()
    return nc


def _get_nc(C):
    key = (C, MM_DTYPE)
    if key not in _nc_cache:
        _nc_cache[key] = _build_nc(C)
    return _nc_cache[key]


def prepare(x, top_weights, top_experts, w1, v1, w2):
    """Host-side routing + sharded input construction.
    Returns (C, in_maps, idx, counts, cw)."""
    x = np.asarray(x, dtype=np.float32)
    top_weights = np.asarray(top_weights, dtype=np.float32)
    top_experts = np.asarray(top_experts).astype(np.int64)
    w1 = np.asarray(w1, dtype=np.float32)
    v1 = np.asarray(v1, dtype=np.float32)
    w2 = np.asarray(w2, dtype=np.float32)

    xf = x.reshape(T, H)

    # combine weights per (token, expert); duplicate slots sum
    cw = np.zeros((T, E), dtype=np.float32)
    np.add.at(cw, (np.arange(T)[:, None], top_experts), top_weights)

    idx = [np.nonzero(cw[:, e])[0] for e in range(E)]
    counts = [len(i) for i in idx]
    C = max(128, -(-max(counts) // P) * P)

    in_maps = []
    for m in range(N_CORES):
        XT = np.zeros((E_LOC, H, C), dtype=np.float32)
        for le in range(E_LOC):
            e = m * E_LOC + le
            XT[le, :, :counts[e]] = xf[idx[e]].T

        def _block(w):
            # [e, F, H] -> [e, ft, p(h%128), o(h//128), f]: each (e, ft)
            # slice contiguous so the DMA runs 128 x 4KB descriptors
            wl = w[m * E_LOC:(m + 1) * E_LOC]
            wl = wl.reshape(E_LOC, FT, P, HT, P)  # [e, ft, f, o, p]
            return np.ascontiguousarray(wl.transpose(0, 1, 4, 3, 2))

        im = {
            "xt": XT,
            "w1t": _block(w1),
            "v1t": _block(v1),
            "w2": np.ascontiguousarray(w2[m * E_LOC:(m + 1) * E_LOC]),
        }
        if MM_DTYPE == "bf16":
            import ml_dtypes
            im = {k: v.astype(ml_dtypes.bfloat16) for k, v in im.items()}
        in_maps.append(im)
    return C, in_maps, idx, counts, cw


def combine(results, idx, counts, cw):
    """Weighted scatter-add of per-core expert outputs into [B, S, H]."""
    out = np.zeros((T, H), dtype=np.float32)
    for m in range(N_CORES):
        ym = results[m]["y"]
        for le in range(E_LOC):
            e = m * E_LOC + le
            n = counts[e]
            if n:
                out[idx[e]] += ym[le, :n, :] * cw[idx[e], e][:, None]
    return out.reshape(B, S, H)


def kernel(x, weights, top_weights, top_experts, w1, v1, w2):
    global LAST_RESULT
    C, in_maps, idx, counts, cw = prepare(
        x, top_weights, top_experts, w1, v1, w2)
    nc = _get_nc(C)
    from concourse.bass_utils import run_bass_kernel_spmd
    res = run_bass_kernel_spmd(nc, in_maps, list(range(N_CORES)), trace=TRACE,
                               trace_cores=TRACE_CORES if TRACE else None)
    LAST_RESULT = res
    return combine(res.results, idx, counts, cw)

